# revision 15
# baseline (speedup 1.0000x reference)
"""LIF spiking-neuron recurrence (nn_LIFSpike) as a Bass/Tile kernel on 8
Trainium2 NeuronCores.

Math (reference): thre = tanh(w); over T=4 steps along the last axis,
    u_t = TAU * u_{t-1} * (1 - o_{t-1}) + x_t
    o_t = (u_t > thre)            # heaviside, output per step
with u_0 = o_0 = 0 and TAU = 0.25.

Bit-exactness: TAU = 0.25 is a power of two and (1 - o) in {0,1}, so
    u_t = fl(0.25 * (u * [u <= thre]) + x_t)
matches the reference exactly. Per step this is two fused DVE
scalar_tensor_tensor ops:
    m  = (u is_le thre) mult u         # reset: u or 0, exact
    u' = (m mult 0.25) add x_t         # leak + input, single rounding
The spike output runs on the Scalar engine as a saturated sigmoid
    o = sigmoid(2^40 * (u - thre))
whose LUT saturates to exactly 0.0 / 1.0 (verified bit-exact vs the
reference on hardware), written directly as uint8 (spikes are {0,1}), which
quarters the output DMA traffic; the host unshard converts back to f32.
Only |u - thre| < ~3e-8 could round the wrong way (~1 element in 3e7,
single-output perturbation only — the recurrence itself uses the exact
is_le comparison).

Sharding/layout: pure elementwise per-neuron -> split the batch dim into 8
chunks of [8,128,32,32,4] (4,194,304 contiguous f32), one per core, no
communication. During host-side sharding each tile's [neurons x 4 steps]
block is transposed to time-planar [4 x neurons] so every device-side access
is unit-stride (interleaved stride-4 access costs ~1.9x on both DVE and
ACT); the output is transposed back during unshard. Tiles are sized
[1024,1024,2048,2048,1024,1024] neurons/partition: small edge tiles shorten
the first-load latency and the final store tail, large middle tiles keep
per-op overhead low. The first tile additionally loads plane-by-plane so
the DVE chain starts after ~0.5 MiB instead of 2 MiB. Loads ride the SP
HWDGE ring; stores ride the ACT HWDGE ring (a store issues right after the
sigmoid that produced its tile, so it never delays a load behind it in a
shared FIFO).
"""

import numpy as np

TAU = 0.25
T = 4
N_CORES = 8
PART = 128
import os as _os

if _os.environ.get("LIF_TILES"):
    TILE_NPP = tuple(int(v) for v in _os.environ["LIF_TILES"].split(","))
else:
    TILE_NPP = (1024,) * 8  # neurons/partition per tile
NPP_TOTAL = sum(TILE_NPP)  # 8192
FULL_SHAPE = (64, 128, 32, 32, T)
CORE_ELEMS = PART * NPP_TOTAL * T  # 4,194,304

SIGMOID_K = float(2.0**40)

_cache: dict = {}


def _build(thre: float, variant: str):
    import concourse.bacc as bacc
    import concourse.mybir as mybir
    from concourse import tile

    f32 = mybir.dt.float32
    u8 = mybir.dt.uint8
    Alu = mybir.AluOpType
    Act = mybir.ActivationFunctionType

    nc = bacc.Bacc("TRN2", target_bir_lowering=False, debug=False)
    # flat per-core DRAM layout: tiles back-to-back, tile i is
    # [PART, T, NPP_i] C-order (partition line = T*NPP_i contiguous f32)
    xd = nc.dram_tensor("x", [CORE_ELEMS], f32, kind="ExternalInput").ap()
    od = nc.dram_tensor("o", [CORE_ELEMS], u8, kind="ExternalOutput").ap()

    with tile.TileContext(nc) as tc:
        with (
            tc.tile_pool(name="const", bufs=1) as cpool,
            tc.tile_pool(name="xp", bufs=3) as xpool,
            tc.tile_pool(name="op", bufs=2) as opool,
            tc.tile_pool(name="work", bufs=3) as work,
        ):
            bias_val = (
                float(-SIGMOID_K * thre) if variant == "sigmoid" else float(-thre)
            )
            bias_t = cpool.tile([PART, 1], f32)
            nc.vector.memset(bias_t[:], bias_val)

            base = 0
            for i, npp in enumerate(TILE_NPP):
                free = npp * T
                xsrc = xd[base : base + PART * free].rearrange(
                    "(p f) -> p f", f=free
                )
                osrc = od[base : base + PART * free].rearrange(
                    "(p f) -> p f", f=free
                )
                base += PART * free

                xt = xpool.tile([PART, free], f32, tag="x")
                if i == 0:
                    # plane-granular first load: compute starts after the
                    # first plane lands instead of the whole tile
                    for t in range(T):
                        nc.sync.dma_start(
                            xt[:, t * npp : (t + 1) * npp],
                            xsrc[:, t * npp : (t + 1) * npp],
                        )
                else:
                    nc.sync.dma_start(xt[:], xsrc)
                ot = opool.tile([PART, free], u8, tag="o")

                u = xt[:, 0:npp]  # u_1 = x_0 (u_0 = o_0 = 0)
                for t in range(T):
                    if variant == "sigmoid":
                        nc.scalar.activation(
                            ot[:, t * npp : (t + 1) * npp],
                            u,
                            Act.Sigmoid,
                            bias=bias_t[:],
                            scale=SIGMOID_K,
                        )
                    else:  # signrelu
                        s = work.tile([PART, npp], f32, tag="s")
                        nc.scalar.activation(s[:], u, Act.Sign, bias=bias_t[:])
                        nc.scalar.activation(
                            ot[:, t * npp : (t + 1) * npp], s[:], Act.Relu
                        )
                    if t < T - 1:
                        m = work.tile([PART, npp], f32, tag="m")
                        nc.vector.scalar_tensor_tensor(
                            m[:], u, thre, u, Alu.is_le, Alu.mult
                        )
                        un = work.tile([PART, npp], f32, tag="u")
                        nc.vector.scalar_tensor_tensor(
                            un[:],
                            m[:],
                            TAU,
                            xt[:, (t + 1) * npp : (t + 2) * npp],
                            Alu.mult,
                            Alu.add,
                        )
                        u = un[:]
                # store on the ACT HWDGE ring: issues right after this tile's
                # last sigmoid in ACT program order, never blocking SP loads
                nc.scalar.dma_start(osrc, ot[:])
    nc.compile()
    return nc


def _get_nc(thre: float, variant: str):
    key = (round(thre, 9), variant)
    if key not in _cache:
        _cache[key] = _build(thre, variant)
    return _cache[key]


def _shard(x: np.ndarray) -> np.ndarray:
    """[64,128,32,32,4] f32 -> [N_CORES, CORE_ELEMS] time-planar per tile."""
    xc = x.reshape(N_CORES, PART, NPP_TOTAL, T)  # [core, part, neuron, t]
    out = np.empty((N_CORES, CORE_ELEMS), np.float32)
    base = 0
    npp_base = 0
    for npp in TILE_NPP:
        blk = xc[:, :, npp_base : npp_base + npp, :]  # [C, P, npp, T]
        blk = blk.transpose(0, 1, 3, 2)  # [C, P, T, npp]
        n = PART * T * npp
        out[:, base : base + n] = blk.reshape(N_CORES, n)
        base += n
        npp_base += npp
    return out


def _unshard(out_planar: np.ndarray) -> np.ndarray:
    """[N_CORES, CORE_ELEMS] uint8 time-planar -> full-shape f32."""
    res = np.empty((N_CORES, PART, NPP_TOTAL, T), np.uint8)
    base = 0
    npp_base = 0
    for npp in TILE_NPP:
        n = PART * T * npp
        blk = out_planar[:, base : base + n].reshape(N_CORES, PART, T, npp)
        res[:, :, npp_base : npp_base + npp, :] = blk.transpose(0, 1, 3, 2)
        base += n
        npp_base += npp
    return res.reshape(FULL_SHAPE).astype(np.float32)


def _run(x_planar, thre: float, variant: str = "sigmoid", **run_kwargs):
    from concourse.bass_utils import run_bass_kernel_spmd

    nc = _get_nc(thre, variant)
    in_maps = [{"x": np.ascontiguousarray(x_planar[c])} for c in range(N_CORES)]
    return run_bass_kernel_spmd(
        nc, in_maps, core_ids=list(range(N_CORES)), **run_kwargs
    )


def kernel(x, w):
    x = np.asarray(x, dtype=np.float32)
    assert x.shape == FULL_SHAPE, x.shape
    thre = float(np.tanh(np.float32(np.asarray(w))))
    xs = _shard(x)
    r = _run(xs, thre)
    out = np.stack([np.asarray(r.results[c]["o"]) for c in range(N_CORES)])
    return _unshard(out)
